# revision 1
# baseline (speedup 1.0000x reference)
"""Multi-head attention (B=2, S=2048, D=1024, H=16, d_head=64) on 8 TRN2 cores.

Sharding: 2-way data parallel over batch x 4-way tensor parallel over heads.
Core c: batch g = c//4, heads [4r, 4r+4) with r = c%4. Each core projects
Q/K/V for its 4 heads from its batch's (pre-transposed) activations, runs
attention per head in a transposed layout (scores^T with keys on partitions),
then AllGathers the per-core head outputs within each 4-core batch group and
computes a 256-row slice of the Wout projection (column parallel). The host
concatenates the per-core output slices.

Layout notes:
  - All matmul operands bf16; accumulation f32 in PSUM.
  - scores^T tiles [128 k, 2048 q] are written to PSUM as bf16 (2 banks,
    non-accumulating) so one ACT exp instruction covers a full k-tile.
  - softmax denominators ride as a 65th "ones" column of V in the PV matmul;
    normalization multiplies by the PE-broadcast reciprocal row.
"""

import os
import sys

import numpy as np

for _p in ("/opt/trn_rl_repo",):
    if _p not in sys.path and os.path.isdir(_p):
        sys.path.append(_p)

import ml_dtypes

import concourse.bacc as bacc
import concourse.bass_utils as _bu
import concourse.mybir as mybir
from concourse.bass_utils import run_bass_kernel_spmd
from concourse.tile import TileContext

# Let walrus dedup LDWEIGHTS for consecutive matmuls that share a stationary
# operand; without it every matmul reloads its weights and the reload gaps
# keep the PE clock throttled.
if not getattr(_bu, "_ldw_opt_patched", False):
    _orig_run_command = _bu.run_command

    def _run_command_ldw(cmd, *a, **kw):
        cmd = [
            c  # ldw-opt=true fails walrus codegen on this BIR; keep default
            if isinstance(c, str)
            else c
            for c in cmd
        ]
        return _orig_run_command(cmd, *a, **kw)

    _bu.run_command = _run_command_ldw
    _bu._ldw_opt_patched = True

P = 128
B, S, DM = 2, 2048, 1024
NH_TOT, EH = 16, 64  # total heads, head dim
NCORES = 8
GROUPS = 2  # batch groups of 4 cores
NH = 4  # heads per core
EHC = NH * EH  # 256: head-concat width per core
NDT = DM // P  # 8 d-tiles
NKT = S // P  # 16 key tiles
QC = 512  # q chunk
NQC = S // QC  # 4
VW = EH + 1  # V width incl. ones column

BF = mybir.dt.bfloat16
F32 = mybir.dt.float32
F32R = mybir.dt.float32r

_cached_nc = None


def build_nc():
    nc = bacc.Bacc("TRN2", target_bir_lowering=False, debug=False, num_devices=NCORES)

    xqt = nc.declare_dram_parameter("xqt", [DM, S], BF, isOutput=False)
    xkt = nc.declare_dram_parameter("xkt", [DM, S], BF, isOutput=False)
    xvt = nc.declare_dram_parameter("xvt", [DM, S], BF, isOutput=False)
    wqt = nc.declare_dram_parameter("wqt", [DM, EHC], BF, isOutput=False)
    wkt = nc.declare_dram_parameter("wkt", [DM, EHC], BF, isOutput=False)
    wvt = nc.declare_dram_parameter("wvt", [DM, EHC], BF, isOutput=False)
    wot = nc.declare_dram_parameter("wot", [DM, EHC], BF, isOutput=False)
    outt = nc.declare_dram_parameter("outt", [EHC, S], F32, isOutput=True)

    with TileContext(nc) as tc:
        with (
            tc.tile_pool(name="persist", bufs=1) as persist,
            tc.tile_pool(name="dram", bufs=1, space="DRAM") as dram,
        ):
            # --- persistent SBUF ---
            wq_sb = persist.tile([P, NDT, EHC], BF)
            wk_sb = persist.tile([P, NDT, EHC], BF)
            wv_sb = persist.tile([P, NDT, EHC], BF)
            wo_sb = persist.tile([P, NDT, EHC], BF)
            for wsb, wpar in ((wq_sb, wqt), (wk_sb, wkt), (wv_sb, wvt), (wo_sb, wot)):
                nc.sync.dma_start(wsb[:], wpar.rearrange("(dt p) e -> p dt e", p=P))

            qt_sb = [persist.tile([P, S], BF, name=f"qt{et}") for et in range(2)]
            kt_sb = [persist.tile([P, S], BF, name=f"kt{et}") for et in range(2)]
            v_sb = persist.tile([P, NKT * NH * VW + P - VW], BF)
            heads_sb = [persist.tile([EH, S], BF, name=f"hd{h}") for h in range(NH)]
            xv_sb = [persist.tile([P, S], BF, name=f"xv{dt}") for dt in range(NDT)]
            nc.gpsimd.memset(v_sb[:], 1.0)  # ones column; V data overwrites cols 0:64

            for dt in range(NDT):
                nc.sync.dma_start(xv_sb[dt][:], xvt[dt * P : (dt + 1) * P, :])

            heads_loc = [
                dram.tile([EHC, QC], BF, name=f"hloc{q4}") for q4 in range(NQC)
            ]
            heads_all = [
                dram.tile([4 * EHC, QC], BF, name=f"hall_d{q4}") for q4 in range(NQC)
            ]

            def emit_allgather(q4):
                nc.gpsimd.collective_compute(
                    "AllGather",
                    mybir.AluOpType.bypass,
                    replica_groups=[[0, 1, 2, 3], [4, 5, 6, 7]],
                    ins=[heads_loc[q4].opt()],
                    outs=[heads_all[q4].opt()],
                )

            # --- V projection (token-major): V = x^T.T @ Wv^T, tok on partitions ---
            with tc.tile_pool(name="vp", bufs=2, space="PSUM") as vp:
                for tt in range(NKT):
                    psv = vp.tile([P, EHC], F32, name="psv")
                    for dt in range(NDT):
                        nc.tensor.matmul(
                            psv[:],
                            xv_sb[dt][:, tt * P : (tt + 1) * P],
                            wv_sb[:, dt, :],
                            start=(dt == 0),
                            stop=(dt == NDT - 1),
                        )
                    nc.vector.tensor_copy(
                        v_sb[:, tt * NH * VW : (tt + 1) * NH * VW].rearrange(
                            "p (h w) -> p h w", w=VW
                        )[:, :, 0:EH],
                        psv[:].rearrange("p (h e) -> p h e", e=EH),
                    )

            # --- Q/K projections: Q^T/K^T = W^T.T @ x^T, e on partitions ---
            with (
                tc.tile_pool(name="xin", bufs=4) as xin,
                tc.tile_pool(name="projp", bufs=1, space="PSUM") as projp,
            ):
                for xpar, wsb, dst in ((xqt, wq_sb, qt_sb), (xkt, wk_sb, kt_sb)):
                    ps = [
                        [
                            projp.tile([P, QC], F32, name=f"pp{et}_{qc}")
                            for qc in range(NQC)
                        ]
                        for et in range(2)
                    ]
                    for dt in range(NDT):
                        xt = xin.tile([P, S], BF, name="xt", tag="xt")
                        nc.sync.dma_start(xt[:], xpar[dt * P : (dt + 1) * P, :])
                        for et in range(2):
                            for qc in range(NQC):
                                nc.tensor.matmul(
                                    ps[et][qc][:],
                                    wsb[:, dt, et * P : (et + 1) * P],
                                    xt[:, qc * QC : (qc + 1) * QC],
                                    start=(dt == 0),
                                    stop=(dt == NDT - 1),
                                )
                    for et in range(2):
                        for qc in range(NQC):
                            nc.vector.tensor_copy(
                                dst[et][:, qc * QC : (qc + 1) * QC], ps[et][qc][:]
                            )

            hall = [
                [persist.tile([P, QC], BF, name=f"hall{q4}_{dt}") for dt in range(NDT)]
                for q4 in range(NQC)
            ]

            def emit_hall_load(q4):
                for dt in range(NDT):
                    # gpsimd queue, emitted between collective triggers so the
                    # wait on AG(q4) never blocks a later trigger that matters
                    nc.gpsimd.dma_start(
                        hall[q4][dt][:], heads_all[q4][dt * P : (dt + 1) * P, :]
                    )

            # --- attention: head pairs (e-tiles), q quarters of 512 ---
            # The two heads of an e-tile compute scores concurrently via PE
            # row tiling (head A on rows 0-63, head B on 64-127) into one
            # [128,1024] PSUM tile (A|B halves) consumed by a single exp.
            # Double-buffered score tiles (4 banks) + 2 PV accumulators
            # (V | ones | junk [128,128] stationary, full-array) = 6 banks,
            # so the exp->scores chain pipelines with no serialization.
            with (
                tc.tile_pool(name="scorep", bufs=2, space="PSUM") as scorep,
                tc.tile_pool(name="pvp", bufs=2, space="PSUM") as pvp,
                tc.tile_pool(name="exps", bufs=4) as expp,
                tc.tile_pool(name="normp", bufs=2) as normp,
            ):

                def normalize(h, q4, pvt):
                    # heads[h][:, q4*512:+512] = pv[0:64] * bcast(1/pv[64]).
                    # The 1-lane denominator row is bounced through DRAM to
                    # spread it across 128 partitions for a fast reciprocal,
                    # then broadcast-read back across 64 partitions.
                    col0 = q4 * QC
                    den = normp.tile([VW, QC], F32, name="den", tag="den")
                    nc.vector.tensor_copy(den[EH : EH + 1, :], pvt[EH : EH + 1, :])
                    den_d = dram.tile([QC], F32, name="den_d", tag="den_d", bufs=2)
                    nc.sync.dma_start(den_d[:], den[EH : EH + 1, :])
                    dsp = normp.tile([P, NQC], F32, name="dsp", tag="dsp")
                    nc.sync.dma_start(dsp[:], den_d[:].rearrange("(p f) -> p f", p=P))
                    rsp = normp.tile([P, NQC], F32, name="rsp", tag="rsp")
                    nc.vector.reciprocal(rsp[:], dsp[:])
                    rcp_d = dram.tile([QC], F32, name="rcp_d", tag="rcp_d", bufs=2)
                    nc.sync.dma_start(rcp_d[:].rearrange("(p f) -> p f", p=P), rsp[:])
                    bc = normp.tile([EH, QC], F32, name="bc", tag="bc")
                    nc.sync.dma_start(bc[:], rcp_d[None, :].to_broadcast([EH, QC]))
                    nc.vector.tensor_mul(
                        heads_sb[h][:, col0 : col0 + QC],
                        pvt[0:EH, :],
                        bc[:],
                    )
                    nc.sync.dma_start(
                        heads_loc[q4][h * EH : (h + 1) * EH, :],
                        heads_sb[h][:, col0 : col0 + QC],
                    )

                def voff(kt, h):
                    return (kt * NH + h) * VW

                # q-quarter outer: after the two e-tile sweeps of a quarter,
                # all four heads have those 512 q columns done and that
                # quarter's AllGather launches, hiding under later sweeps.
                pending = []  # deferred (h, q4, ep, pv_tile) normalizations
                for q4 in range(NQC):
                    q0 = q4 * QC
                    for ep in range(2):
                        hA, hB = 2 * ep, 2 * ep + 1
                        pv = [
                            pvp.tile([P, QC], F32, name=f"pv{lh}", tag=f"pv{lh}")
                            for lh in range(2)
                        ]
                        exring = [None] * NKT
                        for kt in range(NKT + 1):
                            if kt < NKT:
                                # scores + exp for kt, one tile ahead of PV
                                exq = expp.tile([P, 1024], BF, name="exq", tag="exq")
                                exring[kt] = exq
                                s_t = scorep.tile([P, 1024], F32, name="sq", tag="sq")
                                for lh in range(2):
                                    po = lh * EH
                                    nc.tensor.matmul(
                                        s_t[:, lh * QC : (lh + 1) * QC],
                                        kt_sb[ep][po : po + EH, kt * P : (kt + 1) * P],
                                        qt_sb[ep][po : po + EH, q0 : q0 + QC],
                                        start=True,
                                        stop=True,
                                    )
                                nc.scalar.activation(
                                    exq[:],
                                    s_t[:],
                                    mybir.ActivationFunctionType.Exp,
                                    scale=float(1.0 / np.sqrt(EH)),
                                )
                            if kt >= 1:
                                pkt = kt - 1
                                for lh in range(2):
                                    h = hA if lh == 0 else hB
                                    nc.tensor.matmul(
                                        pv[lh][:],
                                        v_sb[:, voff(pkt, h) : voff(pkt, h) + P],
                                        exring[pkt][:, lh * QC : (lh + 1) * QC],
                                        start=(pkt == 0),
                                        stop=(pkt == NKT - 1),
                                        skip_group_check=True,
                                    )
                            if kt == 3 and pending:
                                flush_q4, flush_ep = pending[0][1], pending[0][2]
                                for ph, _, _, ppv in pending:
                                    normalize(ph, flush_q4, ppv)
                                pending = []
                                if flush_ep == 1:
                                    # all four heads done for that quarter
                                    emit_allgather(flush_q4)
                                    if flush_q4 >= 1:
                                        emit_hall_load(flush_q4 - 1)
                        for lh in range(2):
                            pending.append((hA if lh == 0 else hB, q4, ep, pv[lh]))
                flush_q4 = pending[0][1]
                for ph, _, _, ppv in pending:
                    normalize(ph, flush_q4, ppv)
                emit_allgather(NQC - 1)
                emit_hall_load(NQC - 2)
                emit_hall_load(NQC - 1)

            # --- Wout (column-parallel slice): out^T = Wout_slice^T.T @ heads^T ---
            with (
                tc.tile_pool(name="hall", bufs=1) as hallp,
                tc.tile_pool(name="wop", bufs=2, space="PSUM") as wop,
                tc.tile_pool(name="outp", bufs=1) as outp,
            ):
                hall = [
                    [
                        hallp.tile([P, QC], BF, name=f"hall{q4}_{dt}")
                        for dt in range(NDT)
                    ]
                    for q4 in range(NQC)
                ]
                for q4 in range(NQC):
                    for dt in range(NDT):
                        # gpsimd queue: these wait on the collectives and must
                        # not head-of-line-block the sync DMA queue
                        nc.gpsimd.dma_start(
                            hall[q4][dt][:], heads_all[q4][dt * P : (dt + 1) * P, :]
                        )
                out_sb = [outp.tile([P, S], F32, name=f"ot{ot}") for ot in range(2)]
                for q4 in range(NQC):
                    for ot in range(2):
                        pso = wop.tile([P, QC], F32, name="pso", tag="pso")
                        for dt in range(NDT):
                            nc.tensor.matmul(
                                pso[:],
                                wo_sb[:, dt, ot * P : (ot + 1) * P],
                                hall[q4][dt][:, :],
                                start=(dt == 0),
                                stop=(dt == NDT - 1),
                            )
                        nc.vector.tensor_copy(
                            out_sb[ot][:, q4 * QC : (q4 + 1) * QC], pso[:]
                        )
                for ot in range(2):
                    nc.sync.dma_start(outt[ot * P : (ot + 1) * P, :], out_sb[ot][:])

    nc.compile()
    return nc


def _prep_inputs(x_query, x_key, x_value, Wq, Wk, Wv, Wout):
    bf = ml_dtypes.bfloat16
    xt = {}
    for g in range(GROUPS):
        xt[g] = tuple(
            np.ascontiguousarray(np.asarray(x[g], dtype=np.float32).T).astype(bf)
            for x in (x_query, x_key, x_value)
        )
    in_maps = []
    for c in range(NCORES):
        g, r = c // 4, c % 4
        hs = slice(NH * r, NH * (r + 1))
        wq_c = np.ascontiguousarray(
            np.asarray(Wq[hs], dtype=np.float32).reshape(EHC, DM).T
        ).astype(bf)
        wk_c = np.ascontiguousarray(
            np.asarray(Wk[hs], dtype=np.float32).reshape(EHC, DM).T
        ).astype(bf)
        wv_c = np.ascontiguousarray(
            np.asarray(Wv[hs], dtype=np.float32).reshape(EHC, DM).T
        ).astype(bf)
        wo_c = np.ascontiguousarray(
            np.asarray(Wout[EHC * r : EHC * (r + 1), :], dtype=np.float32).T
        ).astype(bf)
        in_maps.append(
            {
                "xqt": xt[g][0],
                "xkt": xt[g][1],
                "xvt": xt[g][2],
                "wqt": wq_c,
                "wkt": wk_c,
                "wvt": wv_c,
                "wot": wo_c,
            }
        )
    return in_maps


def kernel(x_query, x_key, x_value, Wq, Wk, Wv, Wout, _trace=False):
    global _cached_nc
    if _cached_nc is None:
        _cached_nc = build_nc()
    nc = _cached_nc

    in_maps = _prep_inputs(x_query, x_key, x_value, Wq, Wk, Wv, Wout)
    res = run_bass_kernel_spmd(nc, in_maps, list(range(NCORES)), trace=_trace)
    kernel.last_result = res

    out = np.empty((B, S, DM), dtype=np.float32)
    for c in range(NCORES):
        g, r = c // 4, c % 4
        out[g, :, EHC * r : EHC * (r + 1)] = res.results[c]["outt"].T
    return out



# revision 7
# speedup vs baseline: 1.0515x; 1.0515x over previous
"""Multi-head attention (B=2, S=2048, D=1024, H=16, d_head=64) on 8 TRN2 cores.

Sharding: 2-way data parallel over batch x 4-way tensor parallel over heads.
Core c: batch g = c//4, heads [4r, 4r+4) with r = c%4. Each core projects
Q/K/V for its 4 heads from its batch's (pre-transposed) activations, runs
attention per head in a transposed layout (scores^T with keys on partitions),
then AllGathers the per-core head outputs within each 4-core batch group and
computes a 256-row slice of the Wout projection (column parallel). The host
concatenates the per-core output slices.

Schedule (v2): the kernel is paced by the ScalarE exp stream (16.8M exps/core
~ 143us) and the PE matmul stream; everything else hides under them.
  - Ramp: K proj -> V proj -> Q proj (q-chunk 0 only). First exp ~45us.
  - 8 attention sweeps (q-chunk x head-pair): scores (row-tiled head pair),
    exp, PV one tile behind. Q proj chunks 1-3 and Wout chunks 0-2 are
    interleaved into the sweeps' PE slack.
  - PV stationary is [V_h (64 cols) | ones (64 cols)]: the softmax
    denominator lands pre-broadcast on PSUM partitions 64-127 in f32, so
    normalization is just reciprocal + multiply (no DRAM round-trips).
  - Collectives: an AllGather trigger occupies the gpsimd queue until the
    collective completes, so the queue is ordered AG(q), hall-load(q),
    AG(q+1), ... with each hall load a single batched DMA.
"""

import os
import sys

import numpy as np

for _p in ("/opt/trn_rl_repo",):
    if _p not in sys.path and os.path.isdir(_p):
        sys.path.append(_p)

import ml_dtypes

import concourse.bacc as bacc
import concourse.mybir as mybir
from concourse.bass_utils import run_bass_kernel_spmd
from concourse.tile import TileContext

P = 128
B, S, DM = 2, 2048, 1024
NH_TOT, EH = 16, 64  # total heads, head dim
NCORES = 8
GROUPS = 2  # batch groups of 4 cores
NH = 4  # heads per core
EHC = NH * EH  # 256: head-concat width per core
NDT = DM // P  # 8 d-tiles
NKT = S // P  # 16 key tiles
QC = 512  # q chunk
NQC = S // QC  # 4
VW = P  # V block width: 64 V cols + 64 ones cols (den broadcast)

BF = mybir.dt.bfloat16
F32 = mybir.dt.float32

_cached_nc = None


def build_nc():
    nc = bacc.Bacc("TRN2", target_bir_lowering=False, debug=False, num_devices=NCORES)

    xqt = nc.declare_dram_parameter("xqt", [DM, S], BF, isOutput=False)
    xkt = nc.declare_dram_parameter("xkt", [DM, S], BF, isOutput=False)
    xvt = nc.declare_dram_parameter("xvt", [DM, S], BF, isOutput=False)
    wqt = nc.declare_dram_parameter("wqt", [DM, EHC], BF, isOutput=False)
    wkt = nc.declare_dram_parameter("wkt", [DM, EHC], BF, isOutput=False)
    wvt = nc.declare_dram_parameter("wvt", [DM, EHC], BF, isOutput=False)
    wot = nc.declare_dram_parameter("wot", [DM, EHC], BF, isOutput=False)
    outt = nc.declare_dram_parameter("outt", [EHC, S], F32, isOutput=True)

    with TileContext(nc) as tc:
        with (
            tc.tile_pool(name="persist", bufs=1) as persist,
            tc.tile_pool(name="xload", bufs=16) as xload,
            tc.tile_pool(name="dram", bufs=1, space="DRAM") as dram,
        ):
            # --- persistent SBUF ---
            wq_sb = persist.tile([P, NDT, EHC], BF)
            wk_sb = persist.tile([P, NDT, EHC], BF)
            wv_sb = persist.tile([P, NDT, EHC], BF)
            wo_sb = persist.tile([P, NDT, EHC], BF)
            qt_sb = [persist.tile([P, S], BF, name=f"qt{et}") for et in range(2)]
            kt_sb = [persist.tile([P, S], BF, name=f"kt{et}") for et in range(2)]
            v_sb = persist.tile([P, NKT * NH * VW], BF)
            nc.gpsimd.memset(v_sb[:], 1.0)  # ones cols; V data overwrites 0:64

            # input loads: xk+xq stream on the sync queue, weights+xv on the
            # vector queue so the two big streams ride different DMA engines
            nc.scalar.dma_start(
                wk_sb[:], wkt.rearrange("(dt p) e -> p dt e", p=P)
            )
            nc.scalar.dma_start(
                wq_sb[:], wqt.rearrange("(dt p) e -> p dt e", p=P)
            )
            xk = [xload.tile([P, S], BF, name=f"xk{dt}", tag="x") for dt in range(NDT)]
            xv = [xload.tile([P, S], BF, name=f"xv{dt}", tag="x") for dt in range(NDT)]
            xq = [xload.tile([P, S], BF, name=f"xq{dt}", tag="x") for dt in range(NDT)]
            for dt in range(NDT):
                nc.sync.dma_start(xk[dt][:], xkt[dt * P : (dt + 1) * P, :])
            nc.scalar.dma_start(
                wv_sb[:], wvt.rearrange("(dt p) e -> p dt e", p=P)
            )
            for dt in range(NDT):
                # gpsimd queue is idle until the first AllGather (~80us)
                nc.gpsimd.dma_start(xv[dt][:], xvt[dt * P : (dt + 1) * P, :])
            for dt in range(NDT):
                nc.sync.dma_start(xq[dt][:], xqt[dt * P : (dt + 1) * P, :])
            nc.scalar.dma_start(
                wo_sb[:], wot.rearrange("(dt p) e -> p dt e", p=P)
            )

            heads_loc = [
                dram.tile([EHC, QC], BF, name=f"hloc{q4}") for q4 in range(NQC)
            ]
            heads_all = [
                dram.tile([4 * EHC, QC], BF, name=f"hall_d{q4}") for q4 in range(NQC)
            ]

            def emit_allgather(q4):
                nc.gpsimd.collective_compute(
                    "AllGather",
                    mybir.AluOpType.bypass,
                    replica_groups=[[0, 1, 2, 3], [4, 5, 6, 7]],
                    ins=[heads_loc[q4].opt()],
                    outs=[heads_all[q4].opt()],
                )

            # --- K projection: K^T = W^T.T @ x^T, e on partitions ---
            with tc.tile_pool(name="kprojp", bufs=1, space="PSUM") as kprojp:
                psk = [
                    [kprojp.tile([P, QC], F32, name=f"pk{et}_{qc}") for qc in range(NQC)]
                    for et in range(2)
                ]
                for dt in range(NDT):
                    for et in range(2):
                        for qc in range(NQC):
                            nc.tensor.matmul(
                                psk[et][qc][:],
                                wk_sb[:, dt, et * P : (et + 1) * P],
                                xk[dt][:, qc * QC : (qc + 1) * QC],
                                start=(dt == 0),
                                stop=(dt == NDT - 1),
                            )
                for et in range(2):
                    for qc in range(NQC):
                        nc.vector.tensor_copy(
                            kt_sb[et][:, qc * QC : (qc + 1) * QC], psk[et][qc][:]
                        )

            # --- V projection (token-major) + Q projection chunk 0 ---
            with (
                tc.tile_pool(name="vp", bufs=2, space="PSUM") as vp,
                tc.tile_pool(name="qp0", bufs=2, space="PSUM") as qp0,
            ):
                for tt in range(NKT):
                    psv = vp.tile([P, EHC], F32, name="psv", tag="psv")
                    for dt in range(NDT):
                        nc.tensor.matmul(
                            psv[:],
                            xv[dt][:, tt * P : (tt + 1) * P],
                            wv_sb[:, dt, :],
                            start=(dt == 0),
                            stop=(dt == NDT - 1),
                        )
                    nc.vector.tensor_copy(
                        v_sb[:, tt * NH * VW : (tt + 1) * NH * VW].rearrange(
                            "p (h w) -> p h w", w=VW
                        )[:, :, 0:EH],
                        psv[:].rearrange("p (h e) -> p h e", e=EH),
                    )
                for et in range(2):
                    psq = qp0.tile([P, QC], F32, name="psq", tag="psq")
                    for dt in range(NDT):
                        nc.tensor.matmul(
                            psq[:],
                            wq_sb[:, dt, et * P : (et + 1) * P],
                            xq[dt][:, 0:QC],
                            start=(dt == 0),
                            stop=(dt == NDT - 1),
                        )
                    nc.vector.tensor_copy(qt_sb[et][:, 0:QC], psq[:])

            # --- attention sweeps + interleaved Q proj / Wout ---
            with (
                tc.tile_pool(name="scorep", bufs=2, space="PSUM") as scorep,
                tc.tile_pool(name="pvp", bufs=1, space="PSUM") as pvp,
                tc.tile_pool(name="auxp", bufs=1, space="PSUM") as auxp,
                tc.tile_pool(name="wop", bufs=1, space="PSUM") as wop,
                tc.tile_pool(name="exps", bufs=4) as expp,
                tc.tile_pool(name="normp", bufs=4) as normp,
                tc.tile_pool(name="hstp", bufs=8) as hstp,
                tc.tile_pool(name="hallp", bufs=2) as hallp,
                tc.tile_pool(name="outstp", bufs=4) as outstp,
            ):
                hall = [None] * NQC

                def emit_hall_load(q4):
                    hall[q4] = hallp.tile([P, NDT, QC], BF, name="hall", tag="hall")
                    nc.gpsimd.dma_start(
                        hall[q4][:],
                        heads_all[q4].rearrange("(dt p) q -> p dt q", p=P),
                    )

                def normalize(h, q4, pvt):
                    # pvt rows 0:64 = head output, rows 64:128 = denominator
                    # (already broadcast via the 64 ones columns), all f32.
                    col0 = q4 * QC
                    rcp = normp.tile([EH, QC], F32, name="rcp", tag="rcp")
                    nc.vector.reciprocal(rcp[:], pvt[EH : 2 * EH, :])
                    hst = hstp.tile([EH, QC], BF, name="hst", tag="hst")
                    nc.vector.tensor_mul(hst[:], pvt[0:EH, :], rcp[:])
                    nc.sync.dma_start(heads_loc[q4][h * EH : (h + 1) * EH, :], hst[:])

                def emit_qproj(qc, et):
                    # one e-tile chunk of Q proj (8 matmuls, 1 PSUM bank)
                    qp = auxp.tile([P, QC], F32, name="qp", tag="qp")
                    for dt in range(NDT):
                        nc.tensor.matmul(
                            qp[:],
                            wq_sb[:, dt, et * P : (et + 1) * P],
                            xq[dt][:, qc * QC : (qc + 1) * QC],
                            start=(dt == 0),
                            stop=(dt == NDT - 1),
                        )
                    nc.vector.tensor_copy(qt_sb[et][:, qc * QC : (qc + 1) * QC], qp[:])

                def emit_wout(q4, ot):
                    pso = wop.tile([P, QC], F32, name="pso", tag="pso")
                    for dt in range(NDT):
                        nc.tensor.matmul(
                            pso[:],
                            wo_sb[:, dt, ot * P : (ot + 1) * P],
                            hall[q4][:, dt, :],
                            start=(dt == 0),
                            stop=(dt == NDT - 1),
                        )
                    ost = outstp.tile([P, QC], F32, name="ost", tag="ost")
                    nc.vector.tensor_copy(ost[:], pso[:])
                    nc.sync.dma_start(
                        outt[ot * P : (ot + 1) * P, q4 * QC : (q4 + 1) * QC],
                        ost[:],
                    )

                def voff(kt, h):
                    return (kt * NH + h) * VW

                # sweep s = (q4, ep): sweeps 0-2 carry Q proj chunks 1-3;
                # sweeps 4-6 carry Wout chunks 0-2 (their hall data lands
                # during sweeps 3-5); Wout chunk 3 runs in the tail.
                for sweep in range(2 * NQC):
                    q4, ep = sweep // 2, sweep % 2
                    q0 = q4 * QC
                    hA, hB = 2 * ep, 2 * ep + 1
                    pv = [
                        pvp.tile([P, QC], F32, name=f"pv{lh}", tag=f"pv{lh}")
                        for lh in range(2)
                    ]
                    exring = [None] * NKT
                    for kt in range(NKT + 1):
                        if kt < NKT:
                            exq = expp.tile([P, 1024], BF, name="exq", tag="exq")
                            exring[kt] = exq
                            s_t = scorep.tile([P, 1024], F32, name="sq", tag="sq")
                            for lh in range(2):
                                po = lh * EH
                                nc.tensor.matmul(
                                    s_t[:, lh * QC : (lh + 1) * QC],
                                    kt_sb[ep][po : po + EH, kt * P : (kt + 1) * P],
                                    qt_sb[ep][po : po + EH, q0 : q0 + QC],
                                    start=True,
                                    stop=True,
                                )
                            nc.scalar.activation(
                                exq[:],
                                s_t[:],
                                mybir.ActivationFunctionType.Exp,
                                scale=float(1.0 / np.sqrt(EH)),
                            )
                        if kt >= 1:
                            pkt = kt - 1
                            for lh in range(2):
                                h = hA if lh == 0 else hB
                                nc.tensor.matmul(
                                    pv[lh][:],
                                    v_sb[:, voff(pkt, h) : voff(pkt, h) + P],
                                    exring[pkt][:, lh * QC : (lh + 1) * QC],
                                    start=(pkt == 0),
                                    stop=(pkt == NKT - 1),
                                    skip_group_check=True,
                                )
                        # interleave slots: 8 proj/Wout matmuls per half-sweep
                        if kt in (5, 11):
                            half = 0 if kt == 5 else 1
                            if sweep <= 2:
                                emit_qproj(sweep + 1, half)
                            elif 5 <= sweep <= 7:
                                emit_wout(sweep - 5, half)
                    # normalize + ship this sweep's two heads immediately
                    for lh in range(2):
                        normalize(hA if lh == 0 else hB, q4, pv[lh])
                    if ep == 1:
                        emit_allgather(q4)
                        emit_hall_load(q4)

                emit_wout(NQC - 1, 0)
                emit_wout(NQC - 1, 1)

    nc.compile()
    return nc


def _prep_inputs(x_query, x_key, x_value, Wq, Wk, Wv, Wout):
    bf = ml_dtypes.bfloat16
    xt = {}
    for g in range(GROUPS):
        xt[g] = tuple(
            np.ascontiguousarray(np.asarray(x[g], dtype=np.float32).T).astype(bf)
            for x in (x_query, x_key, x_value)
        )
    in_maps = []
    for c in range(NCORES):
        g, r = c // 4, c % 4
        hs = slice(NH * r, NH * (r + 1))
        wq_c = np.ascontiguousarray(
            np.asarray(Wq[hs], dtype=np.float32).reshape(EHC, DM).T
        ).astype(bf)
        wk_c = np.ascontiguousarray(
            np.asarray(Wk[hs], dtype=np.float32).reshape(EHC, DM).T
        ).astype(bf)
        wv_c = np.ascontiguousarray(
            np.asarray(Wv[hs], dtype=np.float32).reshape(EHC, DM).T
        ).astype(bf)
        wo_c = np.ascontiguousarray(
            np.asarray(Wout[EHC * r : EHC * (r + 1), :], dtype=np.float32).T
        ).astype(bf)
        in_maps.append(
            {
                "xqt": xt[g][0],
                "xkt": xt[g][1],
                "xvt": xt[g][2],
                "wqt": wq_c,
                "wkt": wk_c,
                "wvt": wv_c,
                "wot": wo_c,
            }
        )
    return in_maps


def kernel(x_query, x_key, x_value, Wq, Wk, Wv, Wout, _trace=False):
    global _cached_nc
    if _cached_nc is None:
        _cached_nc = build_nc()
    nc = _cached_nc

    in_maps = _prep_inputs(x_query, x_key, x_value, Wq, Wk, Wv, Wout)
    res = run_bass_kernel_spmd(nc, in_maps, list(range(NCORES)), trace=_trace)
    kernel.last_result = res

    out = np.empty((B, S, DM), dtype=np.float32)
    for c in range(NCORES):
        g, r = c // 4, c % 4
        out[g, :, EHC * r : EHC * (r + 1)] = res.results[c]["outt"].T
    return out


# revision 12
# speedup vs baseline: 1.1273x; 1.0720x over previous
"""Multi-head attention (B=2, S=2048, D=1024, H=16, d_head=64) on 8 TRN2 cores.

Sharding: 2-way data parallel over batch x 4-way tensor parallel over heads.
Core c: batch g = c//4, heads [4r, 4r+4) with r = c%4. Each core projects
Q/K/V for its 4 heads from its batch's (pre-transposed) activations, runs
attention per head in a transposed layout (scores^T with keys on partitions),
then AllGathers the per-core head outputs within each 4-core batch group and
computes a 256-row slice of the Wout projection (column parallel). The host
concatenates the per-core output slices.

Schedule (v2): the kernel is paced by the ScalarE exp stream (16.8M exps/core
~ 143us) and the PE matmul stream; everything else hides under them.
  - Ramp: K proj -> V proj -> Q proj (q-chunk 0 only). First exp ~45us.
  - 8 attention sweeps (q-chunk x head-pair): scores (row-tiled head pair),
    exp, PV one tile behind. Q proj chunks 1-3 and Wout chunks 0-2 are
    interleaved into the sweeps' PE slack.
  - PV stationary is [V_h (64 cols) | ones (64 cols)]: the softmax
    denominator lands pre-broadcast on PSUM partitions 64-127 in f32, so
    normalization is just reciprocal + multiply (no DRAM round-trips).
  - Collectives: an AllGather trigger occupies the gpsimd queue until the
    collective completes, so the queue is ordered AG(q), hall-load(q),
    AG(q+1), ... with each hall load a single batched DMA.
"""

import os
import sys

import numpy as np

for _p in ("/opt/trn_rl_repo",):
    if _p not in sys.path and os.path.isdir(_p):
        sys.path.append(_p)

import ml_dtypes

import concourse.bacc as bacc
import concourse.mybir as mybir
from concourse.bass_utils import run_bass_kernel_spmd
from concourse.tile import TileContext

P = 128
B, S, DM = 2, 2048, 1024
NH_TOT, EH = 16, 64  # total heads, head dim
NCORES = 8
GROUPS = 2  # batch groups of 4 cores
NH = 4  # heads per core
EHC = NH * EH  # 256: head-concat width per core
NDT = DM // P  # 8 d-tiles
NKT = S // P  # 16 key tiles
QC = 512  # q chunk
NQC = S // QC  # 4
VW = P  # V block width: 64 V cols + 64 ones cols (den broadcast)

BF = mybir.dt.bfloat16
F32 = mybir.dt.float32

_cached_nc = None


def build_nc():
    nc = bacc.Bacc("TRN2", target_bir_lowering=False, debug=False, num_devices=NCORES)

    xqt = nc.declare_dram_parameter("xqt", [DM, S], BF, isOutput=False)
    xkt = nc.declare_dram_parameter("xkt", [DM, S], BF, isOutput=False)
    xvt = nc.declare_dram_parameter("xvt", [DM, S], BF, isOutput=False)
    # weights arrive host-rearranged to [P, NDT*EHC] so the load is contiguous
    wqt = nc.declare_dram_parameter("wqt", [P, NDT * EHC], BF, isOutput=False)
    wkt = nc.declare_dram_parameter("wkt", [P, NDT * EHC], BF, isOutput=False)
    wvt = nc.declare_dram_parameter("wvt", [P, NDT * EHC], BF, isOutput=False)
    wot = nc.declare_dram_parameter("wot", [P, NDT * EHC], BF, isOutput=False)
    outt = nc.declare_dram_parameter("outt", [EHC, S], F32, isOutput=True)

    with TileContext(nc) as tc:
        with (
            tc.tile_pool(name="persist", bufs=1) as persist,
            tc.tile_pool(name="xload", bufs=16) as xload,
            tc.tile_pool(name="dram", bufs=1, space="DRAM") as dram,
        ):
            # --- persistent SBUF ---
            wq_sb = persist.tile([P, NDT, EHC], BF)
            wk_sb = persist.tile([P, NDT, EHC], BF)
            wv_sb = persist.tile([P, NDT, EHC], BF)
            wo_sb = persist.tile([P, NDT, EHC], BF)
            qt_sb = [persist.tile([P, S], BF, name=f"qt{et}") for et in range(2)]
            kt_sb = [persist.tile([P, S], BF, name=f"kt{et}") for et in range(2)]
            v_sb = persist.tile([P, NKT * NH * VW], BF)
            nc.gpsimd.memset(v_sb[:], 1.0)  # ones cols; V data overwrites 0:64

            # input loads: xk+xq stream on the sync queue, weights+xv on the
            # vector queue so the two big streams ride different DMA engines
            nc.scalar.dma_start(wk_sb[:].rearrange("p d e -> p (d e)"), wkt[:])
            nc.scalar.dma_start(wq_sb[:].rearrange("p d e -> p (d e)"), wqt[:])
            xk = [xload.tile([P, S], BF, name=f"xk{dt}", tag="x") for dt in range(NDT)]
            xv = [xload.tile([P, S], BF, name=f"xv{dt}", tag="x") for dt in range(NDT)]
            xq = [xload.tile([P, S], BF, name=f"xq{dt}", tag="x") for dt in range(NDT)]
            for dt in range(NDT):
                nc.sync.dma_start(xk[dt][:], xkt[dt * P : (dt + 1) * P, :])
            nc.scalar.dma_start(wv_sb[:].rearrange("p d e -> p (d e)"), wvt[:])
            for dt in range(NDT):
                # gpsimd queue is idle until the first AllGather (~80us)
                nc.gpsimd.dma_start(xv[dt][:], xvt[dt * P : (dt + 1) * P, :])
            for dt in range(NDT):
                nc.sync.dma_start(xq[dt][:], xqt[dt * P : (dt + 1) * P, :])
            nc.scalar.dma_start(wo_sb[:].rearrange("p d e -> p (d e)"), wot[:])

            heads_loc = [
                dram.tile([EHC, QC], BF, name=f"hloc{q4}") for q4 in range(NQC)
            ]
            heads_all = [
                dram.tile([4 * EHC, QC], BF, name=f"hall_d{q4}") for q4 in range(NQC)
            ]

            def emit_allgather(q4):
                nc.gpsimd.collective_compute(
                    "AllGather",
                    mybir.AluOpType.bypass,
                    replica_groups=[[0, 1, 2, 3], [4, 5, 6, 7]],
                    ins=[heads_loc[q4].opt()],
                    outs=[heads_all[q4].opt()],
                )

            # --- K projection: K^T = W^T.T @ x^T, e on partitions ---
            with tc.tile_pool(name="kprojp", bufs=1, space="PSUM") as kprojp:
                psk = [
                    [kprojp.tile([P, QC], F32, name=f"pk{et}_{qc}") for qc in range(NQC)]
                    for et in range(2)
                ]
                for dt in range(NDT):
                    for et in range(2):
                        for qc in range(NQC):
                            nc.tensor.matmul(
                                psk[et][qc][:],
                                wk_sb[:, dt, et * P : (et + 1) * P],
                                xk[dt][:, qc * QC : (qc + 1) * QC],
                                start=(dt == 0),
                                stop=(dt == NDT - 1),
                            )
                for et in range(2):
                    for qc in range(NQC):
                        nc.vector.tensor_copy(
                            kt_sb[et][:, qc * QC : (qc + 1) * QC], psk[et][qc][:]
                        )

            # --- V projection (token-major) + Q projection chunk 0 ---
            with (
                tc.tile_pool(name="vp", bufs=2, space="PSUM") as vp,
                tc.tile_pool(name="qp0", bufs=2, space="PSUM") as qp0,
            ):
                for tt in range(NKT):
                    psv = vp.tile([P, EHC], F32, name="psv", tag="psv")
                    for dt in range(NDT):
                        nc.tensor.matmul(
                            psv[:],
                            xv[dt][:, tt * P : (tt + 1) * P],
                            wv_sb[:, dt, :],
                            start=(dt == 0),
                            stop=(dt == NDT - 1),
                        )
                    nc.vector.tensor_copy(
                        v_sb[:, tt * NH * VW : (tt + 1) * NH * VW].rearrange(
                            "p (h w) -> p h w", w=VW
                        )[:, :, 0:EH],
                        psv[:].rearrange("p (h e) -> p h e", e=EH),
                    )
                for et in range(2):
                    psq = qp0.tile([P, QC], F32, name="psq", tag="psq")
                    for dt in range(NDT):
                        nc.tensor.matmul(
                            psq[:],
                            wq_sb[:, dt, et * P : (et + 1) * P],
                            xq[dt][:, 0:QC],
                            start=(dt == 0),
                            stop=(dt == NDT - 1),
                        )
                    nc.vector.tensor_copy(qt_sb[et][:, 0:QC], psq[:])

            # --- attention sweeps + interleaved Q proj / Wout ---
            with (
                tc.tile_pool(name="scorep", bufs=2, space="PSUM") as scorep,
                tc.tile_pool(name="pvp", bufs=1, space="PSUM") as pvp,
                tc.tile_pool(name="auxp", bufs=1, space="PSUM") as auxp,
                tc.tile_pool(name="wop", bufs=1, space="PSUM") as wop,
                tc.tile_pool(name="exps", bufs=4) as expp,
                tc.tile_pool(name="normp", bufs=4) as normp,
                tc.tile_pool(name="hstp", bufs=8) as hstp,
                tc.tile_pool(name="hallp", bufs=2) as hallp,
                tc.tile_pool(name="outstp", bufs=4) as outstp,
            ):
                hall = [None] * NQC

                def emit_hall_load(q4):
                    hall[q4] = hallp.tile([P, NDT, QC], BF, name="hall", tag="hall")
                    for dt in range(NDT):
                        # per-dt flat loads: the rearranged single-DMA variant
                        # is a strided gather and runs ~5x slower
                        nc.gpsimd.dma_start(
                            hall[q4][:, dt, :],
                            heads_all[q4][dt * P : (dt + 1) * P, :],
                        )

                def normalize(h, q4, pvt):
                    # pvt rows 0:64 = head output, rows 64:128 = denominator
                    # (already broadcast via the 64 ones columns), all f32.
                    # Copy PSUM->SBUF first so the accumulator bank frees after
                    # ~0.7us instead of being held through the whole chain.
                    # reciprocal_approx_fast needs partition-0-aligned operands,
                    # so the denominator copy shifts rows 64:128 down to 0:64.
                    num = normp.tile([EH, QC], F32, name="num", tag="num")
                    nc.vector.tensor_copy(num[:], pvt[0:EH, :])
                    den = normp.tile([EH, QC], F32, name="den", tag="den")
                    nc.vector.tensor_copy(den[:], pvt[EH : 2 * EH, :])
                    rcp = normp.tile([EH, QC], F32, name="rcp", tag="rcp")
                    nc.vector.reciprocal_approx_fast(rcp[:], den[:])
                    hst = hstp.tile([EH, QC], BF, name="hst", tag="hst")
                    nc.vector.tensor_mul(hst[:], num[:], rcp[:])
                    nc.sync.dma_start(heads_loc[q4][h * EH : (h + 1) * EH, :], hst[:])

                def emit_qproj(qc, et):
                    # one e-tile chunk of Q proj (8 matmuls, 1 PSUM bank)
                    qp = auxp.tile([P, QC], F32, name="qp", tag="qp")
                    for dt in range(NDT):
                        nc.tensor.matmul(
                            qp[:],
                            wq_sb[:, dt, et * P : (et + 1) * P],
                            xq[dt][:, qc * QC : (qc + 1) * QC],
                            start=(dt == 0),
                            stop=(dt == NDT - 1),
                        )
                    nc.vector.tensor_copy(qt_sb[et][:, qc * QC : (qc + 1) * QC], qp[:])

                def emit_wout(q4, ot):
                    pso = wop.tile([P, QC], F32, name="pso", tag="pso")
                    for dt in range(NDT):
                        nc.tensor.matmul(
                            pso[:],
                            wo_sb[:, dt, ot * P : (ot + 1) * P],
                            hall[q4][:, dt, :],
                            start=(dt == 0),
                            stop=(dt == NDT - 1),
                        )
                    ost = outstp.tile([P, QC], F32, name="ost", tag="ost")
                    nc.vector.tensor_copy(ost[:], pso[:])
                    nc.sync.dma_start(
                        outt[ot * P : (ot + 1) * P, q4 * QC : (q4 + 1) * QC],
                        ost[:],
                    )

                def voff(kt, h):
                    return (kt * NH + h) * VW

                # sweep s = (q4, ep): sweeps 0-2 carry Q proj chunks 1-3;
                # sweeps 4-6 carry Wout chunks 0-2 (their hall data lands
                # during sweeps 3-5); Wout chunk 3 runs in the tail.
                for sweep in range(2 * NQC):
                    q4, ep = sweep // 2, sweep % 2
                    q0 = q4 * QC
                    hA, hB = 2 * ep, 2 * ep + 1
                    pv = [
                        pvp.tile([P, QC], F32, name=f"pv{lh}", tag=f"pv{lh}")
                        for lh in range(2)
                    ]
                    exring = [None] * NKT
                    for kt in range(NKT + 1):
                        if kt < NKT:
                            exq = expp.tile([P, 1024], BF, name="exq", tag="exq")
                            exring[kt] = exq
                            s_t = scorep.tile([P, 1024], F32, name="sq", tag="sq")
                            for lh in range(2):
                                po = lh * EH
                                nc.tensor.matmul(
                                    s_t[:, lh * QC : (lh + 1) * QC],
                                    kt_sb[ep][po : po + EH, kt * P : (kt + 1) * P],
                                    qt_sb[ep][po : po + EH, q0 : q0 + QC],
                                    start=True,
                                    stop=True,
                                )
                            nc.scalar.activation(
                                exq[:],
                                s_t[:],
                                mybir.ActivationFunctionType.Exp,
                                scale=float(1.0 / np.sqrt(EH)),
                            )
                        if kt >= 1:
                            pkt = kt - 1
                            for lh in range(2):
                                h = hA if lh == 0 else hB
                                nc.tensor.matmul(
                                    pv[lh][:],
                                    v_sb[:, voff(pkt, h) : voff(pkt, h) + P],
                                    exring[pkt][:, lh * QC : (lh + 1) * QC],
                                    start=(pkt == 0),
                                    stop=(pkt == NKT - 1),
                                    skip_group_check=True,
                                )
                        # interleave slots: 8 proj/Wout matmuls per half-sweep
                        if kt in (5, 11):
                            half = 0 if kt == 5 else 1
                            if sweep <= 2:
                                emit_qproj(sweep + 1, half)
                            elif 5 <= sweep <= 7:
                                emit_wout(sweep - 5, half)
                    # normalize + ship this sweep's two heads immediately
                    for lh in range(2):
                        normalize(hA if lh == 0 else hB, q4, pv[lh])
                    if ep == 1:
                        emit_allgather(q4)
                        emit_hall_load(q4)

                emit_wout(NQC - 1, 0)
                emit_wout(NQC - 1, 1)

    nc.compile()
    return nc


def _sb_layout(w_dm_e):
    # [DM, EHC] -> [P, NDT*EHC]: partition-major layout so the device DMA
    # into the [P, NDT, EHC] SBUF tile is a single contiguous transfer
    return np.ascontiguousarray(
        w_dm_e.reshape(NDT, P, EHC).transpose(1, 0, 2).reshape(P, NDT * EHC)
    )


def _prep_inputs(x_query, x_key, x_value, Wq, Wk, Wv, Wout):
    bf = ml_dtypes.bfloat16
    xt = {}
    for g in range(GROUPS):
        xt[g] = tuple(
            np.ascontiguousarray(np.asarray(x[g], dtype=np.float32).T).astype(bf)
            for x in (x_query, x_key, x_value)
        )
    in_maps = []
    for c in range(NCORES):
        g, r = c // 4, c % 4
        hs = slice(NH * r, NH * (r + 1))
        wq_c = _sb_layout(
            np.asarray(Wq[hs], dtype=np.float32).reshape(EHC, DM).T
        ).astype(bf)
        wk_c = _sb_layout(
            np.asarray(Wk[hs], dtype=np.float32).reshape(EHC, DM).T
        ).astype(bf)
        wv_c = _sb_layout(
            np.asarray(Wv[hs], dtype=np.float32).reshape(EHC, DM).T
        ).astype(bf)
        wo_c = _sb_layout(
            np.asarray(Wout[EHC * r : EHC * (r + 1), :], dtype=np.float32).T
        ).astype(bf)
        in_maps.append(
            {
                "xqt": xt[g][0],
                "xkt": xt[g][1],
                "xvt": xt[g][2],
                "wqt": wq_c,
                "wkt": wk_c,
                "wvt": wv_c,
                "wot": wo_c,
            }
        )
    return in_maps


def kernel(x_query, x_key, x_value, Wq, Wk, Wv, Wout, _trace=False):
    global _cached_nc
    if _cached_nc is None:
        _cached_nc = build_nc()
    nc = _cached_nc

    in_maps = _prep_inputs(x_query, x_key, x_value, Wq, Wk, Wv, Wout)
    res = run_bass_kernel_spmd(nc, in_maps, list(range(NCORES)), trace=_trace)
    kernel.last_result = res

    out = np.empty((B, S, DM), dtype=np.float32)
    for c in range(NCORES):
        g, r = c // 4, c % 4
        out[g, :, EHC * r : EHC * (r + 1)] = res.results[c]["outt"].T
    return out


# revision 14
# speedup vs baseline: 1.4221x; 1.2615x over previous
"""Multi-head attention (B=2, S=2048, D=1024, H=16, d_head=64) on 8 TRN2 cores.

Sharding: 2-way data parallel over batch x 4-way tensor parallel over heads.
Core c: batch g = c//4, heads [4r, 4r+4) with r = c%4. Each core projects
Q/K/V for its 4 heads from its batch's (pre-transposed) activations, runs
attention for its heads, then computes its row-parallel partial of the Wout
projection (contraction over its own 256 head dims, full 1024 output dims).
The host unshards by summing the 4 partials of each batch group (the
all-reduce of the row-parallel sharding) -- no device collectives at all.

Schedule (v4): the kernel is paced by the ScalarE exp stream (16.8M exps per
core ~ 143us). Everything else is arranged to hide under it:
  - Q proj (chunk 0) and K proj (chunked (et,qc), dt-inner) complete
    incrementally so the first scores/exp fire ~25us in.
  - V proj, Q proj chunks 1-3, and the Wout partials are emitted at demoted
    scheduler priority: the Tile list scheduler slots them into PE gaps.
    PV lags behind exp via a deep exp ring until V tiles land.
  - PV stationary is [V_h (64 cols) | ones (64 cols)]: the softmax
    denominator lands pre-broadcast on PSUM partitions 64-127 in f32;
    normalization = two aligned copies + reciprocal_approx_fast + multiply.
"""

import os
import sys

import numpy as np

for _p in ("/opt/trn_rl_repo",):
    if _p not in sys.path and os.path.isdir(_p):
        sys.path.append(_p)

import ml_dtypes

import concourse.bacc as bacc
import concourse.mybir as mybir
from concourse.bass_utils import run_bass_kernel_spmd
from concourse.tile import TileContext

P = 128
B, S, DM = 2, 2048, 1024
NH_TOT, EH = 16, 64  # total heads, head dim
NCORES = 8
GROUPS = 2  # batch groups of 4 cores
NH = 4  # heads per core
EHC = NH * EH  # 256: head-concat width per core
NDT = DM // P  # 8 d-tiles
NKT = S // P  # 16 key tiles
QC = 512  # q chunk
NQC = S // QC  # 4
VW = P  # V block width: 64 V cols + 64 ones cols (den broadcast)
DEMOTE = 10_000_000  # scheduler priority offset for gap-filler work

BF = mybir.dt.bfloat16
F32 = mybir.dt.float32

_cached_nc = None


def build_nc():
    nc = bacc.Bacc("TRN2", target_bir_lowering=False, debug=False, num_devices=NCORES)

    xqt = nc.declare_dram_parameter("xqt", [DM, S], BF, isOutput=False)
    xkt = nc.declare_dram_parameter("xkt", [DM, S], BF, isOutput=False)
    xvt = nc.declare_dram_parameter("xvt", [DM, S], BF, isOutput=False)
    # weights arrive host-rearranged to partition-major so loads are contiguous
    wqt = nc.declare_dram_parameter("wqt", [P, NDT * EHC], BF, isOutput=False)
    wkt = nc.declare_dram_parameter("wkt", [P, NDT * EHC], BF, isOutput=False)
    wvt = nc.declare_dram_parameter("wvt", [P, NDT * EHC], BF, isOutput=False)
    # Wout rows for this core's 256 head dims, all 1024 output dims
    wo3t = nc.declare_dram_parameter("wo3t", [P, 2 * DM], BF, isOutput=False)
    # row-parallel partial of out^T; host sums the 4 partials per group
    outt = nc.declare_dram_parameter("outt", [DM, S], F32, isOutput=True)

    with TileContext(nc) as tc:
        with (
            tc.tile_pool(name="persist", bufs=1) as persist,
            tc.tile_pool(name="xkp", bufs=8) as xkp,
            tc.tile_pool(name="xqp", bufs=8) as xqp,
            tc.tile_pool(name="xvp", bufs=8) as xvp,
        ):
            # --- persistent SBUF ---
            wq_sb = persist.tile([P, NDT, EHC], BF)
            wk_sb = persist.tile([P, NDT, EHC], BF)
            wv_sb = persist.tile([P, NDT, EHC], BF)
            wo3_sb = persist.tile([P, 2, DM], BF)
            qt_sb = [persist.tile([P, S], BF, name=f"qt{et}") for et in range(2)]
            kt_sb = [persist.tile([P, S], BF, name=f"kt{et}") for et in range(2)]
            v_sb = persist.tile([P, NKT * NH * VW], BF)
            nc.gpsimd.memset(v_sb[:], 1.0)  # ones cols; V data overwrites 0:64

            nc.scalar.dma_start(wk_sb[:].rearrange("p d e -> p (d e)"), wkt[:])
            nc.scalar.dma_start(wq_sb[:].rearrange("p d e -> p (d e)"), wqt[:])
            xk = [xkp.tile([P, S], BF, name=f"xk{dt}", tag="xk") for dt in range(NDT)]
            xv = [xvp.tile([P, S], BF, name=f"xv{dt}", tag="xv") for dt in range(NDT)]
            xq = [xqp.tile([P, S], BF, name=f"xq{dt}", tag="xq") for dt in range(NDT)]
            for dt in range(NDT):
                nc.sync.dma_start(xk[dt][:], xkt[dt * P : (dt + 1) * P, :])
            nc.scalar.dma_start(wv_sb[:].rearrange("p d e -> p (d e)"), wvt[:])
            for dt in range(NDT):
                # gpsimd queue has no other work: dedicate it to the xv stream
                nc.gpsimd.dma_start(xv[dt][:], xvt[dt * P : (dt + 1) * P, :])
            for dt in range(NDT):
                nc.sync.dma_start(xq[dt][:], xqt[dt * P : (dt + 1) * P, :])
            nc.scalar.dma_start(wo3_sb[:].rearrange("p d e -> p (d e)"), wo3t[:])

            with (
                tc.tile_pool(name="pvp", bufs=1, space="PSUM") as pvp,
                tc.tile_pool(name="vp", bufs=1, space="PSUM") as vp,
                tc.tile_pool(name="auxp", bufs=1, space="PSUM") as auxp,
                tc.tile_pool(name="scorep", bufs=2, space="PSUM") as scorep,
                tc.tile_pool(name="exps", bufs=14) as expp,
                tc.tile_pool(name="normp", bufs=2) as normp,
                tc.tile_pool(name="hcp", bufs=4) as hcp,
                tc.tile_pool(name="outstp", bufs=4) as outstp,
            ):

                def emit_qproj(qc, et):
                    # one e-tile chunk of Q proj: 8 matmuls into 1 PSUM bank
                    qp = auxp.tile([P, QC], F32, name="qp", tag="aux")
                    for dt in range(NDT):
                        nc.tensor.matmul(
                            qp[:],
                            wq_sb[:, dt, et * P : (et + 1) * P],
                            xq[dt][:, qc * QC : (qc + 1) * QC],
                            start=(dt == 0),
                            stop=(dt == NDT - 1),
                            skip_group_check=True,
                        )
                    nc.vector.tensor_copy(qt_sb[et][:, qc * QC : (qc + 1) * QC], qp[:])

                # Q proj chunk 0 first: it gates the first scores and only
                # needs the xq stream, so it preempts K-proj leftovers.
                for et in range(2):
                    emit_qproj(0, et)

                # K proj in (et, qc) chunks, dt-inner, so kt_sb completes
                # incrementally (sweep 0 consumes et=0 chunks first).
                for et in range(2):
                    for qc in range(NQC):
                        kp = auxp.tile([P, QC], F32, name="kp", tag="aux")
                        for dt in range(NDT):
                            nc.tensor.matmul(
                                kp[:],
                                wk_sb[:, dt, et * P : (et + 1) * P],
                                xk[dt][:, qc * QC : (qc + 1) * QC],
                                start=(dt == 0),
                                stop=(dt == NDT - 1),
                                skip_group_check=True,
                            )
                        nc.vector.tensor_copy(
                            kt_sb[et][:, qc * QC : (qc + 1) * QC], kp[:]
                        )

                # V proj + Q proj chunks 1-3: demoted priority -> the
                # scheduler slots them into PE gaps under the exp stream.
                with tc.high_priority(offset=-DEMOTE):
                    for tt in range(NKT):
                        psv = vp.tile([P, EHC], F32, name="psv", tag="psv")
                        for dt in range(NDT):
                            nc.tensor.matmul(
                                psv[:],
                                xv[dt][:, tt * P : (tt + 1) * P],
                                wv_sb[:, dt, :],
                                start=(dt == 0),
                                stop=(dt == NDT - 1),
                                skip_group_check=True,
                            )
                        for h in range(NH):
                            nc.vector.tensor_copy(
                                v_sb[
                                    :,
                                    (tt * NH + h) * VW : (tt * NH + h) * VW + EH,
                                ],
                                psv[:, h * EH : (h + 1) * EH],
                            )
                    for qc in range(1, NQC):
                        for et in range(2):
                            emit_qproj(qc, et)

                heads_cat = [None] * (2 * NQC)

                def normalize(lh, pvt, hc):
                    # pvt rows 0:64 = head output, rows 64:128 = denominator
                    # (broadcast via the 64 ones columns), f32. Two aligned
                    # copies free the PSUM bank fast; reciprocal_approx_fast
                    # requires partition-0-aligned operands.
                    num = normp.tile([EH, QC], F32, name="num", tag="num")
                    nc.vector.tensor_copy(num[:], pvt[0:EH, :])
                    den = normp.tile([EH, QC], F32, name="den", tag="den")
                    nc.vector.tensor_copy(den[:], pvt[EH : 2 * EH, :])
                    rcp = normp.tile([EH, QC], F32, name="rcp", tag="rcp")
                    nc.vector.reciprocal_approx_fast(rcp[:], den[:])
                    nc.vector.tensor_mul(
                        hc[lh * EH : (lh + 1) * EH, :], num[:], rcp[:]
                    )

                def emit_wout(q4):
                    # row-parallel partial: out^T[ot*128:+128, q chunk] from
                    # this core's 4 heads (contraction = 2 e-tiles of 128)
                    for ot in range(NDT):
                        pso = auxp.tile([P, QC], F32, name="pso", tag="aux")
                        for ep2 in range(2):
                            nc.tensor.matmul(
                                pso[:],
                                wo3_sb[:, ep2, ot * P : (ot + 1) * P],
                                heads_cat[2 * q4 + ep2][:],
                                start=(ep2 == 0),
                                stop=(ep2 == 1),
                                skip_group_check=True,
                            )
                        ost = outstp.tile([P, QC], F32, name="ost", tag="ost")
                        nc.vector.tensor_copy(ost[:], pso[:])
                        nc.sync.dma_start(
                            outt[ot * P : (ot + 1) * P, q4 * QC : (q4 + 1) * QC],
                            ost[:],
                        )

                def voff(kt, h):
                    return (kt * NH + h) * VW

                for sweep in range(2 * NQC):
                    q4, ep = sweep // 2, sweep % 2
                    q0 = q4 * QC
                    hA, hB = 2 * ep, 2 * ep + 1
                    pv = [
                        pvp.tile([P, QC], F32, name=f"pv{lh}", tag=f"pv{lh}")
                        for lh in range(2)
                    ]
                    exring = [None] * NKT
                    for kt in range(NKT + 1):
                        if kt < NKT:
                            exq = expp.tile([P, 1024], BF, name="exq", tag="exq")
                            exring[kt] = exq
                            s_t = scorep.tile([P, 1024], F32, name="sq", tag="sq")
                            for lh in range(2):
                                po = lh * EH
                                nc.tensor.matmul(
                                    s_t[:, lh * QC : (lh + 1) * QC],
                                    kt_sb[ep][po : po + EH, kt * P : (kt + 1) * P],
                                    qt_sb[ep][po : po + EH, q0 : q0 + QC],
                                    start=True,
                                    stop=True,
                                )
                            nc.scalar.activation(
                                exq[:],
                                s_t[:],
                                mybir.ActivationFunctionType.Exp,
                                scale=float(1.0 / np.sqrt(EH)),
                            )
                        if kt >= 1:
                            pkt = kt - 1
                            for lh in range(2):
                                h = hA if lh == 0 else hB
                                nc.tensor.matmul(
                                    pv[lh][:],
                                    v_sb[:, voff(pkt, h) : voff(pkt, h) + P],
                                    exring[pkt][:, lh * QC : (lh + 1) * QC],
                                    start=(pkt == 0),
                                    stop=(pkt == NKT - 1),
                                    skip_group_check=True,
                                )
                    # normalize this sweep's two heads into a packed
                    # [128, QC] tile (head A rows 0:64, head B rows 64:128)
                    hc = hcp.tile([P, QC], BF, name="hc", tag="hc")
                    heads_cat[sweep] = hc
                    for lh in range(2):
                        normalize(lh, pv[lh], hc)
                    if ep == 1:
                        with tc.high_priority(offset=-DEMOTE):
                            emit_wout(q4)

    nc.compile()
    return nc


def _sb_layout(w_dm_e, blocks):
    # [blocks*P, E] -> [P, blocks*E] partition-major so the device DMA into a
    # [P, blocks, E] SBUF tile is one contiguous transfer
    e = w_dm_e.shape[1]
    return np.ascontiguousarray(
        w_dm_e.reshape(blocks, P, e).transpose(1, 0, 2).reshape(P, blocks * e)
    )


def _prep_inputs(x_query, x_key, x_value, Wq, Wk, Wv, Wout):
    bf = ml_dtypes.bfloat16
    xt = {}
    for g in range(GROUPS):
        xt[g] = tuple(
            np.ascontiguousarray(np.asarray(x[g], dtype=np.float32).T).astype(bf)
            for x in (x_query, x_key, x_value)
        )
    in_maps = []
    for c in range(NCORES):
        g, r = c // 4, c % 4
        hs = slice(NH * r, NH * (r + 1))
        wq_c = _sb_layout(
            np.asarray(Wq[hs], dtype=np.float32).reshape(EHC, DM).T, NDT
        ).astype(bf)
        wk_c = _sb_layout(
            np.asarray(Wk[hs], dtype=np.float32).reshape(EHC, DM).T, NDT
        ).astype(bf)
        wv_c = _sb_layout(
            np.asarray(Wv[hs], dtype=np.float32).reshape(EHC, DM).T, NDT
        ).astype(bf)
        # Wout rows for my head dims: [EHC, DM], partition-major over 2 e-tiles
        wo3_c = _sb_layout(
            np.ascontiguousarray(
                np.asarray(Wout[:, EHC * r : EHC * (r + 1)], dtype=np.float32).T
            ),
            2,
        ).astype(bf)
        in_maps.append(
            {
                "xqt": xt[g][0],
                "xkt": xt[g][1],
                "xvt": xt[g][2],
                "wqt": wq_c,
                "wkt": wk_c,
                "wvt": wv_c,
                "wo3t": wo3_c,
            }
        )
    return in_maps


def kernel(x_query, x_key, x_value, Wq, Wk, Wv, Wout, _trace=False):
    global _cached_nc
    if _cached_nc is None:
        _cached_nc = build_nc()
    nc = _cached_nc

    in_maps = _prep_inputs(x_query, x_key, x_value, Wq, Wk, Wv, Wout)
    res = run_bass_kernel_spmd(nc, in_maps, list(range(NCORES)), trace=_trace)
    kernel.last_result = res

    out = np.empty((B, S, DM), dtype=np.float32)
    for g in range(GROUPS):
        acc = res.results[4 * g]["outt"].astype(np.float32).copy()
        for r in range(1, 4):
            acc += res.results[4 * g + r]["outt"]
        out[g] = acc.T
    return out
